# revision 21
# baseline (speedup 1.0000x reference)
"""Trainium2 Bass kernel for nn_ExplicitLiePE.

Computes y[b,s] = expm(sum_k r[b,s,k] * skew(L_k)) @ P_sp @ x[b,s] for
B=8, S=1024, d_h=64, d_c=3, on 8 NeuronCores.

Math: A(r) is skew-symmetric (imaginary spectrum), so the expm action on a
vector is evaluated with a Chebyshev/Bessel expansion
    exp(A) x = J_0(t) x + sum_{n>=1} J_n(t) D_n,
    D_0 = 2 x, D_1 = 2 B x, D_{n+1} = 2 B D_n + D_{n-1},  B = A / t,
which needs only matvecs with B (no scaling-and-squaring, no solves) and is
numerically stable because spec(B) lies in i[-1,1] where all Chebyshev states
stay bounded.  B v = (1/t) sum_k r_k (Lsk_k v) batches across all (b,s) pairs
as three shared-weight matmuls plus per-column scalings.

Sharding: pairs (b,s) are flattened, sorted by a cheap rigorous upper bound
on the spectral radius, split into a low band and a high band with their own
polynomial degree, and distributed 1024 pairs per core (512 from each band)
so every core runs the identical SPMD program.
"""

import numpy as np
from contextlib import ExitStack

import concourse.bass as bass
import concourse.tile as tile
from concourse import bacc, mybir
from concourse.bass_utils import run_bass_kernel_spmd

B, S, DH, DC = 8, 1024, 64, 3
NCORES = 8
NPAIRS = B * S
PER_CORE = NPAIRS // NCORES          # 1024
F = PER_CORE // 2 // 2               # 256 free columns per packed chunk
CHUNK_PAIRS = 2 * F                  # 512 pairs per chunk, 2 chunks per core
TAIL_TOL = 1.0e-3

FP16 = mybir.dt.float16
F32 = mybir.dt.float32
F32R = mybir.dt.float32r


# ----------------------------------------------------------------- host math
def _bessel_j(nmax: int, theta: float) -> np.ndarray:
    """J_0..J_nmax via Miller's downward recurrence (no scipy dependency)."""
    m = nmax + 40 + int(theta)
    j = np.zeros(m + 2, dtype=np.float64)
    j[m] = 1e-30
    for n in range(m, 0, -1):
        j[n - 1] = 2.0 * n / theta * j[n] - j[n + 1]
        if abs(j[n - 1]) > 1e10:
            j[: m + 2] /= 1e10
    s = j[0] + 2.0 * np.sum(j[2:m:2])
    return j[: nmax + 1] / s


def _degree_for(theta: float, tol: float) -> int:
    jj = np.abs(_bessel_j(int(theta) + 45, max(theta, 0.25)))
    for m in range(max(2, int(theta)), int(theta) + 41):
        if 2.0 * jj[m + 1 : m + 12].sum() < tol:
            return max(m, 2)
    return int(theta) + 40


def _plan(r_flat: np.ndarray, lsk: np.ndarray):
    """Rigorous per-pair upper bound on rho(sum_k r_k Lsk_k), 2-band split."""
    rho = np.linalg.svd(lsk, compute_uv=False)[:, 0]                # [3]
    prod2 = np.einsum("kij,ljm->klim", np.swapaxes(lsk, 1, 2), lsk)  # LkT Ll
    q2 = np.linalg.svd(prod2.reshape(9, DH, DH), compute_uv=False)[:, 0].reshape(3, 3)
    prod4 = np.einsum("klim,pqmj->klpqij", prod2, prod2)
    q4 = np.linalg.svd(prod4.reshape(81, DH, DH), compute_uv=False)[:, 0].reshape(3, 3, 3, 3)
    rf = r_flat.astype(np.float64)
    b1 = rf @ rho
    b2 = np.sqrt(np.einsum("nk,kl,nl->n", rf, q2, rf))
    b4 = np.einsum("nk,nl,nm,np,klmp->n", rf, rf, rf, rf, q4) ** 0.25
    b = np.minimum(np.minimum(b1, b2), b4) * 1.002 + 1e-3
    order = np.argsort(b, kind="stable")
    half = NPAIRS // 2
    theta_lo = max(float(b[order[:half]].max()), 0.25)
    theta_hi = max(float(b[order[half:]].max()), 0.25)
    m_lo = _degree_for(theta_lo, TAIL_TOL)
    m_hi = _degree_for(theta_hi, TAIL_TOL)
    return order, (theta_lo, theta_hi), (m_lo, m_hi)


# ------------------------------------------------------------- bass program
def _build_program(m_lo: int, m_hi: int, theta_lo: float, theta_hi: float):
    degrees = (m_lo, m_hi)
    inv_thetas = (1.0 / theta_lo, 1.0 / theta_hi)
    nc = bacc.Bacc("TRN2", debug=False, num_devices=NCORES)

    xs = nc.dram_tensor("xs", [PER_CORE, DH], F32R, kind="ExternalInput").ap()
    rr = nc.dram_tensor("rr", [2, DC, 2, F], FP16, kind="ExternalInput").ap()
    lmats = nc.dram_tensor("lmats", [2, 128, DC * 128], F32, kind="ExternalInput").ap()
    wpsp = nc.dram_tensor("wpsp", [128, 128], F32R, kind="ExternalInput").ap()
    n_wacc = (m_lo + 1) + (m_hi + 1) + 1
    wacc = nc.dram_tensor("wacc", [128, n_wacc * 128], FP16, kind="ExternalInput").ap()
    ones2 = nc.dram_tensor("ones2", [2, 128], FP16, kind="ExternalInput").ap()
    ident = nc.dram_tensor("ident", [128, 128], F32R, kind="ExternalInput").ap()
    ys = nc.dram_tensor("ys", [PER_CORE, DH], F32R, kind="ExternalOutput").ap()

    with tile.TileContext(nc) as tc, ExitStack() as ctx:
        const = ctx.enter_context(tc.tile_pool(name="const", bufs=1))
        work = ctx.enter_context(tc.tile_pool(name="work", bufs=2))
        state = ctx.enter_context(tc.tile_pool(name="state", bufs=4))
        psum_d = ctx.enter_context(tc.tile_pool(name="psum_d", bufs=1, space="PSUM"))
        psum_t = ctx.enter_context(tc.tile_pool(name="psum_t", bufs=2, space="PSUM"))

        # ---- shared constants
        id_sb = const.tile([128, 128], F32R)
        nc.sync.dma_start(id_sb[:], ident[:])
        ones2_sb = const.tile([2, 128], FP16)
        nc.sync.dma_start(ones2_sb[:], ones2[:])
        wpsp_sb = const.tile([128, 128], F32R)
        nc.sync.dma_start(wpsp_sb[:], wpsp[:])
        wacc_sb = const.tile([128, n_wacc * 128], FP16)
        nc.sync.dma_start(wacc_sb[:], wacc[:])

        # ---- weights W_k = (L^T - L) = 2*Lsk^T, host-shipped in blockdiag
        # layout; skew computed on device with one subtract (off-blocks 0-0=0)
        lm_sb = const.tile([128, 2 * DC * 128], F32)
        nc.sync.dma_start(lm_sb[:, : DC * 128], lmats[0])
        nc.sync.dma_start(lm_sb[:, DC * 128 :], lmats[1])
        wsub = const.tile([128, DC * 128], F32)
        nc.vector.tensor_sub(wsub[:], lm_sb[:, DC * 128 :], lm_sb[:, : DC * 128])
        w_cat = const.tile([128, DC * 128], FP16)
        nc.vector.tensor_copy(w_cat[:], wsub[:])

        # ---- phase 1: prologues (transpose/pack x, P_sp apply, Rb build)
        st_sb = [None, None]
        rb_cats = [None, None]
        d_banks = [None, None]
        acc_banks = [None, None]
        for c in range(2):
            xt_ps = psum_t.tile([DH, 4 * 128], F32R, tag="tmp")
            for t in range(4):
                x_raw = work.tile([128, DH], F32R, tag="xraw")
                nc.sync.dma_start(
                    x_raw[:], xs[c * CHUNK_PAIRS + t * 128 : c * CHUNK_PAIRS + (t + 1) * 128, :]
                )
                nc.tensor.transpose(
                    xt_ps[:, t * 128 : (t + 1) * 128], x_raw[:], id_sb[:]
                )
            xt_sb = work.tile([DH, 4 * 128], F32R, tag="xtsb")
            nc.scalar.copy(xt_sb[:], xt_ps[:])
            x_pk = work.tile([128, F], F32R, tag="xpk")
            nc.sync.dma_start(x_pk[:DH, :], xt_sb[:, :F])
            nc.sync.dma_start(x_pk[DH:, :], xt_sb[:, F:])

            xh_ps = psum_t.tile([128, F], F32, tag="tmp")
            nc.tensor.matmul(xh_ps[:], wpsp_sb[:], x_pk[:], start=True, stop=True)
            st = state.tile([128, F], FP16, tag=f"st{c}")
            nc.scalar.copy(st[:], xh_ps[:])
            st_sb[c] = st

            rb_cat = const.tile([128, DC * F], FP16, tag=f"rb{c}")
            for k in range(DC):
                rrow = work.tile([2, F], FP16, tag="rrow")
                nc.sync.dma_start(rrow[:], rr[c, k])
                rb_ps = psum_t.tile([128, F], F32, tag="tmp")
                nc.tensor.matmul(rb_ps[:], ones2_sb[:], rrow[:], start=True, stop=True)
                nc.scalar.activation(
                    rb_cat[:, k * F : (k + 1) * F],
                    rb_ps[:],
                    mybir.ActivationFunctionType.Copy,
                    scale=float(inv_thetas[c]),
                )
            rb_cats[c] = rb_cat

            d_even = psum_d.tile([128, F], F32, tag=f"de{c}")
            d_odd = psum_d.tile([128, F], F32, tag=f"do{c}")
            acc_ps = psum_d.tile([128, F], F32, tag=f"acc{c}")
            base = 1 + (0 if c == 0 else m_lo + 1)
            nc.tensor.matmul(d_even[:], wacc_sb[:, 0:128], st[:], start=True, stop=True,
                             skip_group_check=True)
            nc.tensor.matmul(
                acc_ps[:], wacc_sb[:, base * 128 : (base + 1) * 128], st[:],
                start=True, stop=False, skip_group_check=True,
            )
            d_banks[c] = [d_even, d_odd]
            acc_banks[c] = acc_ps

        # ---- phase 2: both Chebyshev recurrences, interleaved by step.
        # The u-multiply is one fused DVE op; for n>=2 it reads D_{n-1}
        # straight from PSUM (fp32 - keeps the recurrence input exact and
        # keeps ScalarE off the critical chain; the fp16 SBUF copies only
        # feed the accumulator matmul).
        bases = [1, 1 + (m_lo + 1)]
        for n in range(1, max(degrees) + 1):
            for c in range(2):
                m_c = degrees[c]
                if n > m_c:
                    continue
                rb_cat = rb_cats[c]
                u_cat = work.tile([128, DC * F], FP16, tag=f"u{c}")
                if n == 1:
                    src = st_sb[c][:]          # xhat (D_0/2) in fp16 SBUF
                else:
                    src = d_banks[c][(n - 1) % 2][:]
                nc.vector.tensor_mul(
                    u_cat[:].rearrange("p (k f) -> p k f", k=DC),
                    src.unsqueeze(1).broadcast_to([128, DC, F]),
                    rb_cat[:].rearrange("p (k f) -> p k f", k=DC),
                )
                d_cur = d_banks[c][n % 2]
                for k in range(DC):
                    nc.tensor.matmul(
                        d_cur[:],
                        w_cat[:, k * 128 : (k + 1) * 128],
                        u_cat[:, k * F : (k + 1) * F],
                        start=(n == 1 and k == 0),
                        stop=(n == m_c or n == m_c - 1) and k == DC - 1,
                        skip_group_check=True,
                    )
                st = state.tile([128, F], FP16, tag=f"st{c}")
                nc.scalar.copy(st[:], d_cur[:])
                st_sb[c] = st
                nc.tensor.matmul(
                    acc_banks[c][:],
                    wacc_sb[:, (bases[c] + n) * 128 : (bases[c] + n + 1) * 128],
                    st[:],
                    start=False,
                    stop=(n == m_c),
                    skip_group_check=True,
                )

        # ---- phase 3: epilogues
        for c in range(2):
            acc_sb = work.tile([128, F], F32R, tag="accsb")
            nc.scalar.copy(acc_sb[:], acc_banks[c][:])
            for t in range(4):
                half, col = divmod(t, 2)
                y_ps = psum_t.tile([128, DH], F32R, tag="tmp")
                nc.tensor.transpose(
                    y_ps[:],
                    acc_sb[half * DH : (half + 1) * DH, col * 128 : (col + 1) * 128],
                    id_sb[half * DH : (half + 1) * DH, half * DH : (half + 1) * DH],
                )
                y_sb = work.tile([128, DH], F32R, tag="ysb")
                nc.scalar.copy(y_sb[:], y_ps[:])
                row0 = c * CHUNK_PAIRS + half * 256 + col * 128
                nc.sync.dma_start(ys[row0 : row0 + 128, :], y_sb[:])

    nc.compile()
    return nc


_PROGRAM_CACHE: dict = {}


def _get_program(m_lo: int, m_hi: int, theta_lo: float, theta_hi: float):
    key = (m_lo, m_hi, round(theta_lo, 9), round(theta_hi, 9))
    if key not in _PROGRAM_CACHE:
        _PROGRAM_CACHE[key] = _build_program(m_lo, m_hi, theta_lo, theta_hi)
    return _PROGRAM_CACHE[key]


# ------------------------------------------------------------------- driver
def kernel(x, r_grid, L_param, P_sp):
    x = np.asarray(x, dtype=np.float32)
    r_grid = np.asarray(r_grid, dtype=np.float32)
    L_param = np.asarray(L_param, dtype=np.float32)
    P_sp = np.asarray(P_sp, dtype=np.float32)

    xf = x.reshape(NPAIRS, DH)
    rf = r_grid.reshape(NPAIRS, DC)
    lsk = 0.5 * (L_param - np.swapaxes(L_param, 1, 2))

    order, thetas, (m_lo, m_hi) = _plan(rf, lsk)
    half = NPAIRS // 2
    bands = [order[:half], order[half:]]

    # shared constants
    def _blk(mats):  # [3,64,64] -> [128, 3*128] blockdiag placement
        out = np.zeros((128, DC * 128), np.float32)
        for k in range(DC):
            out[:DH, k * 128 : k * 128 + DH] = mats[k]
            out[DH:, k * 128 + DH : (k + 1) * 128] = mats[k]
        return out

    lmats = np.stack(
        [_blk(L_param), _blk(np.swapaxes(L_param, 1, 2))]
    ).astype(np.float32)
    wpsp = np.zeros((128, 128), np.float32)
    wpsp[:DH, :DH] = P_sp.T
    wpsp[DH:, DH:] = P_sp.T
    eye128 = np.eye(128, dtype=np.float32)
    j_lo = _bessel_j(m_lo, thetas[0])
    j_hi = _bessel_j(m_hi, thetas[1])
    wacc = np.concatenate(
        [
            2.0 * eye128[None],
            j_lo[:, None, None] * eye128[None],
            j_hi[:, None, None] * eye128[None],
        ]
    ).astype(np.float16)
    wacc = np.ascontiguousarray(np.transpose(wacc, (1, 0, 2)).reshape(128, -1))
    ones2 = np.zeros((2, 128), np.float16)
    ones2[0, :DH] = 1.0
    ones2[1, DH:] = 1.0

    in_maps = []
    core_pairs = []
    for core in range(NCORES):
        idx = np.concatenate(
            [bands[0][core * CHUNK_PAIRS : (core + 1) * CHUNK_PAIRS],
             bands[1][core * CHUNK_PAIRS : (core + 1) * CHUNK_PAIRS]]
        )
        core_pairs.append(idx)
        rrc = np.empty((2, DC, 2, F), np.float16)
        for c in range(2):
            rc = rf[idx[c * CHUNK_PAIRS : (c + 1) * CHUNK_PAIRS]]  # [512, 3]
            for k in range(DC):
                rrc[c, k, 0] = rc[:F, k].astype(np.float16)
                rrc[c, k, 1] = rc[F:, k].astype(np.float16)
        in_maps.append(
            {
                "xs": xf[idx].copy(),
                "rr": rrc,
                "lmats": lmats,
                "wpsp": wpsp,
                "wacc": wacc,
                "ones2": ones2,
                "ident": eye128,
            }
        )

    nc = _get_program(m_lo, m_hi, thetas[0], thetas[1])
    res = run_bass_kernel_spmd(nc, in_maps, core_ids=list(range(NCORES)))

    y = np.empty((NPAIRS, DH), np.float32)
    for core in range(NCORES):
        y[core_pairs[core]] = res.results[core]["ys"]
    return y.reshape(B, S, DH)


# revision 27
# speedup vs baseline: 1.3138x; 1.3138x over previous
"""Trainium2 Bass kernel for nn_ExplicitLiePE.

Computes y[b,s] = expm(sum_k r[b,s,k] * skew(L_k)) @ P_sp @ x[b,s] for
B=8, S=1024, d_h=64, d_c=3, on 8 NeuronCores.

Math: A(r) is skew-symmetric (imaginary spectrum), so the expm action on a
vector is evaluated with a Chebyshev/Bessel expansion
    exp(A) x = J_0(t) x + sum_{n>=1} J_n(t) D_n,
    D_0 = 2 x, D_1 = 2 B x, D_{n+1} = 2 B D_n + D_{n-1},  B = A / t,
which needs only matvecs with B (no scaling-and-squaring, no solves) and is
numerically stable because spec(B) lies in i[-1,1] where all Chebyshev states
stay bounded.  B v = (1/t) sum_k r_k (Lsk_k v) batches across all (b,s) pairs
as three shared-weight matmuls plus per-column scalings.

Sharding: pairs (b,s) are flattened, sorted by a cheap rigorous upper bound
on the spectral radius, split into a low band and a high band with their own
polynomial degree, and distributed 1024 pairs per core (512 from each band)
so every core runs the identical SPMD program.
"""

import numpy as np
from contextlib import ExitStack

import concourse.bass as bass
import concourse.tile as tile
from concourse import bacc, mybir
from concourse.bass_utils import run_bass_kernel_spmd

B, S, DH, DC = 8, 1024, 64, 3
NCORES = 8
NPAIRS = B * S
PER_CORE = NPAIRS // NCORES          # 1024
F = PER_CORE // 2 // 2               # 256 free columns per packed chunk
CHUNK_PAIRS = 2 * F                  # 512 pairs per chunk, 2 chunks per core
TAIL_TOL = 1.0e-3

FP16 = mybir.dt.float16
F32 = mybir.dt.float32
F32R = mybir.dt.float32r


# ----------------------------------------------------------------- host math
def _bessel_j(nmax: int, theta: float) -> np.ndarray:
    """J_0..J_nmax via Miller's downward recurrence (no scipy dependency)."""
    m = nmax + 40 + int(theta)
    j = np.zeros(m + 2, dtype=np.float64)
    j[m] = 1e-30
    for n in range(m, 0, -1):
        j[n - 1] = 2.0 * n / theta * j[n] - j[n + 1]
        if abs(j[n - 1]) > 1e10:
            j[: m + 2] /= 1e10
    s = j[0] + 2.0 * np.sum(j[2:m:2])
    return j[: nmax + 1] / s


def _degree_for(theta: float, tol: float) -> int:
    jj = np.abs(_bessel_j(int(theta) + 45, max(theta, 0.25)))
    for m in range(max(2, int(theta)), int(theta) + 41):
        if 2.0 * jj[m + 1 : m + 12].sum() < tol:
            return max(m, 2)
    return int(theta) + 40


def _plan(r_flat: np.ndarray, lsk: np.ndarray):
    """Rigorous per-pair upper bound on rho(sum_k r_k Lsk_k), 2-band split."""
    rho = np.linalg.svd(lsk, compute_uv=False)[:, 0]                # [3]
    prod2 = np.einsum("kij,ljm->klim", np.swapaxes(lsk, 1, 2), lsk)  # LkT Ll
    q2 = np.linalg.svd(prod2.reshape(9, DH, DH), compute_uv=False)[:, 0].reshape(3, 3)
    prod4 = np.einsum("klim,pqmj->klpqij", prod2, prod2)
    q4 = np.linalg.svd(prod4.reshape(81, DH, DH), compute_uv=False)[:, 0].reshape(3, 3, 3, 3)
    rf = r_flat.astype(np.float64)
    b1 = rf @ rho
    b2 = np.sqrt(np.einsum("nk,kl,nl->n", rf, q2, rf))
    b4 = np.einsum("nk,nl,nm,np,klmp->n", rf, rf, rf, rf, q4) ** 0.25
    b = np.minimum(np.minimum(b1, b2), b4) * 1.002 + 1e-3
    # Uniform degree: the per-step latency chain means the slowest chunk sets
    # the wall clock, so adaptive per-band degrees do not pay; both chunks use
    # the global bound.  (order kept as identity.)
    order = np.arange(NPAIRS)
    theta = max(float(b.max()), 0.25)
    m = _degree_for(theta, TAIL_TOL)
    return order, (theta, theta), (m, m)


# ------------------------------------------------------------- bass program
def _build_program(m_lo: int, m_hi: int, theta_lo: float, theta_hi: float):
    assert m_lo == m_hi and theta_lo == theta_hi
    m = m_lo
    inv_theta = 1.0 / theta_lo
    nc = bacc.Bacc("TRN2", debug=False, num_devices=NCORES)

    xs = nc.dram_tensor("xs", [PER_CORE, DH], F32R, kind="ExternalInput").ap()
    rr = nc.dram_tensor("rr", [2, DC, 2, F], FP16, kind="ExternalInput").ap()
    lmats = nc.dram_tensor("lmats", [2, 128, DC * 128], F32, kind="ExternalInput").ap()
    wpsp = nc.dram_tensor("wpsp", [128, 128], F32R, kind="ExternalInput").ap()
    n_wacc = m + 2  # [2I, J_0*I .. J_m*I]
    wacc = nc.dram_tensor("wacc", [128, n_wacc * 128], FP16, kind="ExternalInput").ap()
    ones2 = nc.dram_tensor("ones2", [2, 128], FP16, kind="ExternalInput").ap()
    ident = nc.dram_tensor("ident", [128, 128], F32R, kind="ExternalInput").ap()
    ys = nc.dram_tensor("ys", [PER_CORE, DH], F32R, kind="ExternalOutput").ap()

    with tile.TileContext(nc) as tc, ExitStack() as ctx:
        const = ctx.enter_context(tc.tile_pool(name="const", bufs=1))
        work = ctx.enter_context(tc.tile_pool(name="work", bufs=2))
        state = ctx.enter_context(tc.tile_pool(name="state", bufs=4))
        psum_d = ctx.enter_context(tc.tile_pool(name="psum_d", bufs=1, space="PSUM"))
        psum_t = ctx.enter_context(tc.tile_pool(name="psum_t", bufs=2, space="PSUM"))

        # ---- shared constants
        id_sb = const.tile([128, 128], F32R)
        nc.sync.dma_start(id_sb[:], ident[:])
        ones2_sb = const.tile([2, 128], FP16)
        nc.sync.dma_start(ones2_sb[:], ones2[:])
        wpsp_sb = const.tile([128, 128], F32R)
        nc.sync.dma_start(wpsp_sb[:], wpsp[:])
        wacc_sb = const.tile([128, n_wacc * 128], FP16)
        nc.sync.dma_start(wacc_sb[:], wacc[:])

        # ---- weights W_k = (L^T - L) = 2*Lsk^T, host-shipped in blockdiag
        # layout; skew computed on device with one subtract (off-blocks 0-0=0)
        lm_sb = const.tile([128, 2 * DC * 128], F32)
        nc.sync.dma_start(lm_sb[:, : DC * 128], lmats[0])
        nc.sync.dma_start(lm_sb[:, DC * 128 :], lmats[1])
        wsub = const.tile([128, DC * 128], F32)
        nc.vector.tensor_sub(wsub[:], lm_sb[:, DC * 128 :], lm_sb[:, : DC * 128])
        w_cat = const.tile([128, DC * 128], FP16)
        nc.vector.tensor_copy(w_cat[:], wsub[:])

        # ---- phase 1: prologues (transpose/pack x, P_sp apply, Rb build)
        st_sb = [None, None]
        rb_cats = [None, None]
        d_banks = [None, None]
        acc_banks = [None, None]
        for c in range(2):
            x_in = work.tile([128, 4 * DH], F32R, tag="xin")
            nc.sync.dma_start(
                x_in[:].rearrange("p (t h) -> p t h", t=4),
                xs[c * CHUNK_PAIRS : (c + 1) * CHUNK_PAIRS, :].rearrange(
                    "(t p) h -> p t h", p=128
                ),
            )
            xt_ps = psum_t.tile([DH, 4 * 128], F32R, tag="tmp")
            for t in range(4):
                nc.tensor.transpose(
                    xt_ps[:, t * 128 : (t + 1) * 128],
                    x_in[:, t * DH : (t + 1) * DH],
                    id_sb[:],
                )
            xt_sb = work.tile([DH, 4 * 128], F32R, tag="xtsb")
            nc.scalar.copy(xt_sb[:], xt_ps[:])
            x_pk = work.tile([128, F], F32R, tag="xpk")
            nc.sync.dma_start(x_pk[:DH, :], xt_sb[:, :F])
            nc.sync.dma_start(x_pk[DH:, :], xt_sb[:, F:])

            xh_ps = psum_t.tile([128, F], F32, tag="tmp")
            nc.tensor.matmul(xh_ps[:], wpsp_sb[:], x_pk[:], start=True, stop=True)
            st = state.tile([128, F], FP16, tag=f"st{c}")
            nc.scalar.copy(st[:], xh_ps[:])
            st_sb[c] = st

            rr_sb = work.tile([2, DC * F], FP16, tag="rrow")
            nc.sync.dma_start(
                rr_sb[:].rearrange("g (k f) -> g k f", k=DC), rr[c].rearrange("k g f -> g k f")
            )
            rb_cat = const.tile([128, DC * F], FP16, tag=f"rb{c}")
            for k in range(DC):
                rb_ps = psum_t.tile([128, F], F32, tag="tmp")
                nc.tensor.matmul(
                    rb_ps[:], ones2_sb[:], rr_sb[:, k * F : (k + 1) * F],
                    start=True, stop=True,
                )
                nc.scalar.activation(
                    rb_cat[:, k * F : (k + 1) * F],
                    rb_ps[:],
                    mybir.ActivationFunctionType.Copy,
                    scale=float(inv_theta),
                )
            rb_cats[c] = rb_cat

            d_even = psum_d.tile([128, F], F32, tag=f"de{c}")
            d_odd = psum_d.tile([128, F], F32, tag=f"do{c}")
            acc_ps = psum_d.tile([128, F], F32, tag=f"acc{c}")
            nc.tensor.matmul(d_even[:], wacc_sb[:, 0:128], st[:], start=True, stop=True,
                             skip_group_check=True)
            nc.tensor.matmul(
                acc_ps[:], wacc_sb[:, 128:256], st[:],
                start=True, stop=False, skip_group_check=True,
            )
            d_banks[c] = [d_even, d_odd]
            acc_banks[c] = acc_ps

        # ---- phase 2: both Chebyshev recurrences, interleaved by step.
        # Per chunk-step chain: PE (3 blockdiag matmuls accumulating onto
        # D_{n-2}) -> ACT (fp16 copy of D_n) -> DVE (one fused 2x-mode
        # multiply producing all three scaled inputs) -> PE.  Two equal-depth
        # streams keep all three engines busy.
        for n in range(1, m + 1):
            for c in range(2):
                rb_cat = rb_cats[c]
                u_cat = work.tile([128, DC * F], FP16, tag=f"u{c}")
                nc.vector.tensor_mul(
                    u_cat[:].rearrange("p (k f) -> p k f", k=DC),
                    st_sb[c][:].unsqueeze(1).broadcast_to([128, DC, F]),
                    rb_cat[:].rearrange("p (k f) -> p k f", k=DC),
                )
                d_cur = d_banks[c][n % 2]
                for k in range(DC):
                    nc.tensor.matmul(
                        d_cur[:],
                        w_cat[:, k * 128 : (k + 1) * 128],
                        u_cat[:, k * F : (k + 1) * F],
                        start=(n == 1 and k == 0),
                        stop=(n == m or n == m - 1) and k == DC - 1,
                        skip_group_check=True,
                    )
                st = state.tile([128, F], FP16, tag=f"st{c}")
                nc.scalar.copy(st[:], d_cur[:])
                st_sb[c] = st
                nc.tensor.matmul(
                    acc_banks[c][:],
                    wacc_sb[:, (1 + n) * 128 : (2 + n) * 128],
                    st[:],
                    start=False,
                    stop=(n == m),
                    skip_group_check=True,
                )

        # ---- phase 3: epilogues (transpose back, single copy + DMA per chunk)
        for c in range(2):
            acc_sb = work.tile([128, F], F32R, tag="accsb")
            nc.scalar.copy(acc_sb[:], acc_banks[c][:])
            for t in range(4):
                half, col = divmod(t, 2)
                y_ps = psum_t.tile([128, DH], F32R, tag="tmp")
                nc.tensor.transpose(
                    y_ps[:],
                    acc_sb[half * DH : (half + 1) * DH, col * 128 : (col + 1) * 128],
                    id_sb[half * DH : (half + 1) * DH, half * DH : (half + 1) * DH],
                )
                y_sb = work.tile([128, DH], F32R, tag="ysb")
                nc.scalar.copy(y_sb[:], y_ps[:])
                row0 = c * CHUNK_PAIRS + half * 256 + col * 128
                nc.sync.dma_start(ys[row0 : row0 + 128, :], y_sb[:])

    nc.compile()
    return nc


_PROGRAM_CACHE: dict = {}


def _get_program(m_lo: int, m_hi: int, theta_lo: float, theta_hi: float):
    key = (m_lo, m_hi, round(theta_lo, 9), round(theta_hi, 9))
    if key not in _PROGRAM_CACHE:
        _PROGRAM_CACHE[key] = _build_program(m_lo, m_hi, theta_lo, theta_hi)
    return _PROGRAM_CACHE[key]


# ------------------------------------------------------------------- driver
def kernel(x, r_grid, L_param, P_sp):
    x = np.asarray(x, dtype=np.float32)
    r_grid = np.asarray(r_grid, dtype=np.float32)
    L_param = np.asarray(L_param, dtype=np.float32)
    P_sp = np.asarray(P_sp, dtype=np.float32)

    xf = x.reshape(NPAIRS, DH)
    rf = r_grid.reshape(NPAIRS, DC)
    lsk = 0.5 * (L_param - np.swapaxes(L_param, 1, 2))

    order, thetas, (m_lo, m_hi) = _plan(rf, lsk)
    half = NPAIRS // 2
    bands = [order[:half], order[half:]]

    # shared constants
    def _blk(mats):  # [3,64,64] -> [128, 3*128] blockdiag placement
        out = np.zeros((128, DC * 128), np.float32)
        for k in range(DC):
            out[:DH, k * 128 : k * 128 + DH] = mats[k]
            out[DH:, k * 128 + DH : (k + 1) * 128] = mats[k]
        return out

    lmats = np.stack(
        [_blk(L_param), _blk(np.swapaxes(L_param, 1, 2))]
    ).astype(np.float32)
    wpsp = np.zeros((128, 128), np.float32)
    wpsp[:DH, :DH] = P_sp.T
    wpsp[DH:, DH:] = P_sp.T
    eye128 = np.eye(128, dtype=np.float32)
    j_lo = _bessel_j(m_lo, thetas[0])
    j_hi = _bessel_j(m_hi, thetas[1])
    wacc = np.concatenate(
        [2.0 * eye128[None], j_lo[:, None, None] * eye128[None]]
    ).astype(np.float16)
    wacc = np.ascontiguousarray(np.transpose(wacc, (1, 0, 2)).reshape(128, -1))
    ones2 = np.zeros((2, 128), np.float16)
    ones2[0, :DH] = 1.0
    ones2[1, DH:] = 1.0

    in_maps = []
    core_pairs = []
    for core in range(NCORES):
        idx = np.concatenate(
            [bands[0][core * CHUNK_PAIRS : (core + 1) * CHUNK_PAIRS],
             bands[1][core * CHUNK_PAIRS : (core + 1) * CHUNK_PAIRS]]
        )
        core_pairs.append(idx)
        rrc = np.empty((2, DC, 2, F), np.float16)
        for c in range(2):
            rc = rf[idx[c * CHUNK_PAIRS : (c + 1) * CHUNK_PAIRS]]  # [512, 3]
            for k in range(DC):
                rrc[c, k, 0] = rc[:F, k].astype(np.float16)
                rrc[c, k, 1] = rc[F:, k].astype(np.float16)
        in_maps.append(
            {
                "xs": xf[idx].copy(),
                "rr": rrc,
                "lmats": lmats,
                "wpsp": wpsp,
                "wacc": wacc,
                "ones2": ones2,
                "ident": eye128,
            }
        )

    nc = _get_program(m_lo, m_hi, thetas[0], thetas[1])
    res = run_bass_kernel_spmd(nc, in_maps, core_ids=list(range(NCORES)))

    y = np.empty((NPAIRS, DH), np.float32)
    for core in range(NCORES):
        y[core_pairs[core]] = res.results[core]["ys"]
    return y.reshape(B, S, DH)


# revision 29
# speedup vs baseline: 1.3695x; 1.0424x over previous
"""Trainium2 Bass kernel for nn_ExplicitLiePE.

Computes y[b,s] = expm(sum_k r[b,s,k] * skew(L_k)) @ P_sp @ x[b,s] for
B=8, S=1024, d_h=64, d_c=3, on 8 NeuronCores.

Math: A(r) is skew-symmetric (imaginary spectrum), so the expm action on a
vector is evaluated with a Chebyshev/Bessel expansion
    exp(A) x = J_0(t) x + sum_{n>=1} J_n(t) D_n,
    D_0 = 2 x, D_1 = 2 B x, D_{n+1} = 2 B D_n + D_{n-1},  B = A / t,
which needs only matvecs with B (no scaling-and-squaring, no solves) and is
numerically stable because spec(B) lies in i[-1,1] where all Chebyshev states
stay bounded.  B v = (1/t) sum_k r_k (Lsk_k v) batches across all (b,s) pairs
as three shared-weight matmuls plus per-column scalings.

Sharding: pairs (b,s) are flattened, sorted by a cheap rigorous upper bound
on the spectral radius, split into a low band and a high band with their own
polynomial degree, and distributed 1024 pairs per core (512 from each band)
so every core runs the identical SPMD program.
"""

import numpy as np
from contextlib import ExitStack

import concourse.bass as bass
import concourse.tile as tile
from concourse import bacc, mybir
from concourse.bass_utils import run_bass_kernel_spmd

B, S, DH, DC = 8, 1024, 64, 3
NCORES = 8
NPAIRS = B * S
PER_CORE = NPAIRS // NCORES          # 1024
F = PER_CORE // 2 // 2               # 256 free columns per packed chunk
CHUNK_PAIRS = 2 * F                  # 512 pairs per chunk, 2 chunks per core
TAIL_TOL = 1.0e-3

FP16 = mybir.dt.float16
F32 = mybir.dt.float32
F32R = mybir.dt.float32r


# ----------------------------------------------------------------- host math
def _bessel_j(nmax: int, theta: float) -> np.ndarray:
    """J_0..J_nmax via Miller's downward recurrence (no scipy dependency)."""
    m = nmax + 40 + int(theta)
    j = np.zeros(m + 2, dtype=np.float64)
    j[m] = 1e-30
    for n in range(m, 0, -1):
        j[n - 1] = 2.0 * n / theta * j[n] - j[n + 1]
        if abs(j[n - 1]) > 1e10:
            j[: m + 2] /= 1e10
    s = j[0] + 2.0 * np.sum(j[2:m:2])
    return j[: nmax + 1] / s


def _degree_for(theta: float, tol: float) -> int:
    jj = np.abs(_bessel_j(int(theta) + 45, max(theta, 0.25)))
    for m in range(max(2, int(theta)), int(theta) + 41):
        if 2.0 * jj[m + 1 : m + 12].sum() < tol:
            return max(m, 2)
    return int(theta) + 40


def _plan(r_flat: np.ndarray, lsk: np.ndarray):
    """Rigorous per-pair upper bound on rho(sum_k r_k Lsk_k), 2-band split."""
    rho = np.linalg.svd(lsk, compute_uv=False)[:, 0]                # [3]
    prod2 = np.einsum("kij,ljm->klim", np.swapaxes(lsk, 1, 2), lsk)  # LkT Ll
    q2 = np.linalg.svd(prod2.reshape(9, DH, DH), compute_uv=False)[:, 0].reshape(3, 3)
    prod4 = np.einsum("klim,pqmj->klpqij", prod2, prod2)
    q4 = np.linalg.svd(prod4.reshape(81, DH, DH), compute_uv=False)[:, 0].reshape(3, 3, 3, 3)
    rf = r_flat.astype(np.float64)
    b1 = rf @ rho
    b2 = np.sqrt(np.einsum("nk,kl,nl->n", rf, q2, rf))
    b4 = np.einsum("nk,nl,nm,np,klmp->n", rf, rf, rf, rf, q4) ** 0.25
    b = np.minimum(np.minimum(b1, b2), b4) * 1.002 + 1e-3
    # Uniform degree: the per-step latency chain means the slowest chunk sets
    # the wall clock, so adaptive per-band degrees do not pay; both chunks use
    # the global bound.  (order kept as identity.)
    order = np.arange(NPAIRS)
    theta = max(float(b.max()), 0.25)
    m = _degree_for(theta, TAIL_TOL)
    return order, (theta, theta), (m, m)


# ------------------------------------------------------------- bass program
def _build_program(m_lo: int, m_hi: int, theta_lo: float, theta_hi: float):
    assert m_lo == m_hi and theta_lo == theta_hi
    m = m_lo
    inv_theta = 1.0 / theta_lo
    nc = bacc.Bacc("TRN2", debug=False, num_devices=NCORES)

    xs = nc.dram_tensor("xs", [PER_CORE, DH], F32R, kind="ExternalInput").ap()
    rr = nc.dram_tensor("rr", [2, DC, 2, F], FP16, kind="ExternalInput").ap()
    lmats = nc.dram_tensor("lmats", [2, 128, DC * 128], F32, kind="ExternalInput").ap()
    wpsp = nc.dram_tensor("wpsp", [128, 128], F32R, kind="ExternalInput").ap()
    n_wacc = m + 2  # [2I, J_0*I .. J_m*I]
    wacc = nc.dram_tensor("wacc", [128, n_wacc * 128], FP16, kind="ExternalInput").ap()
    ones2 = nc.dram_tensor("ones2", [2, 128], FP16, kind="ExternalInput").ap()
    ident = nc.dram_tensor("ident", [128, 128], F32R, kind="ExternalInput").ap()
    ys = nc.dram_tensor("ys", [2, 128, 4 * DH], F32R, kind="ExternalOutput").ap()

    with tile.TileContext(nc) as tc, ExitStack() as ctx:
        const = ctx.enter_context(tc.tile_pool(name="const", bufs=1))
        work = ctx.enter_context(tc.tile_pool(name="work", bufs=2))
        state = ctx.enter_context(tc.tile_pool(name="state", bufs=4))
        psum_d = ctx.enter_context(tc.tile_pool(name="psum_d", bufs=1, space="PSUM"))
        psum_t = ctx.enter_context(tc.tile_pool(name="psum_t", bufs=2, space="PSUM"))

        # ---- shared constants
        id_sb = const.tile([128, 128], F32R)
        nc.sync.dma_start(id_sb[:], ident[:])
        ones2_sb = const.tile([2, 128], FP16)
        nc.sync.dma_start(ones2_sb[:], ones2[:])
        wpsp_sb = const.tile([128, 128], F32R)
        nc.sync.dma_start(wpsp_sb[:], wpsp[:])
        wacc_sb = const.tile([128, n_wacc * 128], FP16)
        nc.sync.dma_start(wacc_sb[:], wacc[:])

        # ---- weights W_k = (L^T - L) = 2*Lsk^T, host-shipped in blockdiag
        # layout; skew computed on device with one subtract (off-blocks 0-0=0)
        lm_sb = const.tile([128, 2 * DC * 128], F32)
        nc.sync.dma_start(lm_sb[:, : DC * 128], lmats[0])
        nc.sync.dma_start(lm_sb[:, DC * 128 :], lmats[1])
        wsub = const.tile([128, DC * 128], F32)
        nc.vector.tensor_sub(wsub[:], lm_sb[:, DC * 128 :], lm_sb[:, : DC * 128])
        w_cat = const.tile([128, DC * 128], FP16)
        nc.vector.tensor_copy(w_cat[:], wsub[:])

        # ---- phase 1: prologues (transpose/pack x, P_sp apply, Rb build)
        st_sb = [None, None]
        rb_cats = [None, None]
        d_banks = [None, None]
        acc_banks = [None, None]
        for c in range(2):
            x_in = work.tile([128, 4 * DH], F32R, tag="xin")
            nc.sync.dma_start(
                x_in[:].rearrange("p (t h) -> p t h", t=4),
                xs[c * CHUNK_PAIRS : (c + 1) * CHUNK_PAIRS, :].rearrange(
                    "(t p) h -> p t h", p=128
                ),
            )
            xt_ps = psum_t.tile([DH, 4 * 128], F32R, tag="tmp")
            for t in range(4):
                nc.tensor.transpose(
                    xt_ps[:, t * 128 : (t + 1) * 128],
                    x_in[:, t * DH : (t + 1) * DH],
                    id_sb[:],
                )
            xt_sb = work.tile([DH, 4 * 128], F32R, tag="xtsb")
            nc.scalar.copy(xt_sb[:], xt_ps[:])
            x_pk = work.tile([128, F], F32R, tag="xpk")
            nc.sync.dma_start(x_pk[:DH, :], xt_sb[:, :F])
            nc.sync.dma_start(x_pk[DH:, :], xt_sb[:, F:])

            xh_ps = psum_t.tile([128, F], F32, tag="tmp")
            nc.tensor.matmul(xh_ps[:], wpsp_sb[:], x_pk[:], start=True, stop=True)
            st = state.tile([128, F], FP16, tag=f"st{c}")
            nc.scalar.copy(st[:], xh_ps[:])
            st_sb[c] = st

            rr_sb = work.tile([2, DC * F], FP16, tag="rrow")
            nc.sync.dma_start(
                rr_sb[:].rearrange("g (k f) -> g k f", k=DC), rr[c].rearrange("k g f -> g k f")
            )
            rb_cat = const.tile([128, DC * F], FP16, tag=f"rb{c}")
            for k in range(DC):
                rb_ps = psum_t.tile([128, F], F32, tag="tmp")
                nc.tensor.matmul(
                    rb_ps[:], ones2_sb[:], rr_sb[:, k * F : (k + 1) * F],
                    start=True, stop=True,
                )
                nc.scalar.activation(
                    rb_cat[:, k * F : (k + 1) * F],
                    rb_ps[:],
                    mybir.ActivationFunctionType.Copy,
                    scale=float(inv_theta),
                )
            rb_cats[c] = rb_cat

            d_even = psum_d.tile([128, F], F32, tag=f"de{c}")
            d_odd = psum_d.tile([128, F], F32, tag=f"do{c}")
            acc_ps = psum_d.tile([128, F], F32, tag=f"acc{c}")
            nc.tensor.matmul(d_even[:], wacc_sb[:, 0:128], st[:], start=True, stop=True,
                             skip_group_check=True)
            nc.tensor.matmul(
                acc_ps[:], wacc_sb[:, 128:256], st[:],
                start=True, stop=False, skip_group_check=True,
            )
            d_banks[c] = [d_even, d_odd]
            acc_banks[c] = acc_ps

        # ---- phase 2: both Chebyshev recurrences, interleaved by step.
        # Per chunk-step chain: PE (3 blockdiag matmuls accumulating onto
        # D_{n-2}) -> ACT (fp16 copy of D_n) -> DVE (one fused 2x-mode
        # multiply producing all three scaled inputs) -> PE.  Two equal-depth
        # streams keep all three engines busy.
        for n in range(1, m + 1):
            for c in range(2):
                rb_cat = rb_cats[c]
                u_cat = work.tile([128, DC * F], FP16, tag=f"u{c}")
                nc.vector.tensor_mul(
                    u_cat[:].rearrange("p (k f) -> p k f", k=DC),
                    st_sb[c][:].unsqueeze(1).broadcast_to([128, DC, F]),
                    rb_cat[:].rearrange("p (k f) -> p k f", k=DC),
                )
                d_cur = d_banks[c][n % 2]
                for k in range(DC):
                    nc.tensor.matmul(
                        d_cur[:],
                        w_cat[:, k * 128 : (k + 1) * 128],
                        u_cat[:, k * F : (k + 1) * F],
                        start=(n == 1 and k == 0),
                        stop=(n == m or n == m - 1) and k == DC - 1,
                        skip_group_check=True,
                    )
                st = state.tile([128, F], FP16, tag=f"st{c}")
                nc.scalar.copy(st[:], d_cur[:])
                st_sb[c] = st
                nc.tensor.matmul(
                    acc_banks[c][:],
                    wacc_sb[:, (1 + n) * 128 : (2 + n) * 128],
                    st[:],
                    start=False,
                    stop=(n == m),
                    skip_group_check=True,
                )

        # ---- phase 3: epilogues (transpose back, single copy + DMA per chunk)
        for c in range(2):
            acc_sb = work.tile([128, F], F32R, tag="accsb")
            nc.scalar.copy(acc_sb[:], acc_banks[c][:])
            y_sb = work.tile([128, 4 * DH], F32R, tag="ysb")
            for t in range(4):
                half, col = divmod(t, 2)
                y_ps = psum_t.tile([128, DH], F32R, tag="tmp")
                nc.tensor.transpose(
                    y_ps[:],
                    acc_sb[half * DH : (half + 1) * DH, col * 128 : (col + 1) * 128],
                    id_sb[half * DH : (half + 1) * DH, half * DH : (half + 1) * DH],
                )
                nc.scalar.copy(y_sb[:, t * DH : (t + 1) * DH], y_ps[:])
            nc.sync.dma_start(ys[c], y_sb[:])

    nc.compile()
    return nc


_PROGRAM_CACHE: dict = {}


def _get_program(m_lo: int, m_hi: int, theta_lo: float, theta_hi: float):
    key = (m_lo, m_hi, round(theta_lo, 9), round(theta_hi, 9))
    if key not in _PROGRAM_CACHE:
        _PROGRAM_CACHE[key] = _build_program(m_lo, m_hi, theta_lo, theta_hi)
    return _PROGRAM_CACHE[key]


# ------------------------------------------------------------------- driver
def kernel(x, r_grid, L_param, P_sp):
    x = np.asarray(x, dtype=np.float32)
    r_grid = np.asarray(r_grid, dtype=np.float32)
    L_param = np.asarray(L_param, dtype=np.float32)
    P_sp = np.asarray(P_sp, dtype=np.float32)

    xf = x.reshape(NPAIRS, DH)
    rf = r_grid.reshape(NPAIRS, DC)
    lsk = 0.5 * (L_param - np.swapaxes(L_param, 1, 2))

    order, thetas, (m_lo, m_hi) = _plan(rf, lsk)
    half = NPAIRS // 2
    bands = [order[:half], order[half:]]

    # shared constants
    def _blk(mats):  # [3,64,64] -> [128, 3*128] blockdiag placement
        out = np.zeros((128, DC * 128), np.float32)
        for k in range(DC):
            out[:DH, k * 128 : k * 128 + DH] = mats[k]
            out[DH:, k * 128 + DH : (k + 1) * 128] = mats[k]
        return out

    lmats = np.stack(
        [_blk(L_param), _blk(np.swapaxes(L_param, 1, 2))]
    ).astype(np.float32)
    wpsp = np.zeros((128, 128), np.float32)
    wpsp[:DH, :DH] = P_sp.T
    wpsp[DH:, DH:] = P_sp.T
    eye128 = np.eye(128, dtype=np.float32)
    j_lo = _bessel_j(m_lo, thetas[0])
    j_hi = _bessel_j(m_hi, thetas[1])
    wacc = np.concatenate(
        [2.0 * eye128[None], j_lo[:, None, None] * eye128[None]]
    ).astype(np.float16)
    wacc = np.ascontiguousarray(np.transpose(wacc, (1, 0, 2)).reshape(128, -1))
    ones2 = np.zeros((2, 128), np.float16)
    ones2[0, :DH] = 1.0
    ones2[1, DH:] = 1.0

    in_maps = []
    core_pairs = []
    for core in range(NCORES):
        idx = np.concatenate(
            [bands[0][core * CHUNK_PAIRS : (core + 1) * CHUNK_PAIRS],
             bands[1][core * CHUNK_PAIRS : (core + 1) * CHUNK_PAIRS]]
        )
        core_pairs.append(idx)
        rrc = np.empty((2, DC, 2, F), np.float16)
        for c in range(2):
            rc = rf[idx[c * CHUNK_PAIRS : (c + 1) * CHUNK_PAIRS]]  # [512, 3]
            for k in range(DC):
                rrc[c, k, 0] = rc[:F, k].astype(np.float16)
                rrc[c, k, 1] = rc[F:, k].astype(np.float16)
        in_maps.append(
            {
                "xs": xf[idx].copy(),
                "rr": rrc,
                "lmats": lmats,
                "wpsp": wpsp,
                "wacc": wacc,
                "ones2": ones2,
                "ident": eye128,
            }
        )

    nc = _get_program(m_lo, m_hi, thetas[0], thetas[1])
    res = run_bass_kernel_spmd(nc, in_maps, core_ids=list(range(NCORES)))

    y = np.empty((NPAIRS, DH), np.float32)
    for core in range(NCORES):
        yc = res.results[core]["ys"].reshape(2, 128, 4, DH)
        yc = np.transpose(yc, (0, 2, 1, 3)).reshape(PER_CORE, DH)
        y[core_pairs[core]] = yc
    return y.reshape(B, S, DH)
